# revision 12
# baseline (speedup 1.0000x reference)
"""Trainium2 Bass kernel for nn_HandGNNEncoder (2-layer GCN on 21-node hand
graphs + mean pool), data-parallel over 8 NeuronCores.

Math restructure (exact):
  reference: h1 = relu(A @ (x @ W1) + b1); out = mean_t(A @ (h1 @ W2) + b2)
  mean-pool is linear, so with m[s] = column-mean of A (all > 0):
      out[g] = sum_s m[s] * h1[g,s,:] @ W2 + b2
  m[s] > 0 folds inside the relu:  m*relu(z) = relu(m*z).
  Stage 1 (PE): z[(s,f), g] = TW.T @ x'[g]   with TW[(s',c),(s,f)] =
      m[s]*A[s,s']*W1[c,f], bias row via a constant-1 input row, plus one
      extra column that relu's to the constant 1 (carries b2 in stage 2).
  Stage 2 (PE): out[d, g] = sum_k W2R_k.T @ relu_k  accumulated in PSUM.

Layouts are feature-major with graphs streaming as the matmul moving dim;
host does all transposes so every DMA is contiguous.
"""

import numpy as np

import concourse.bass as bass
import concourse.mybir as mybir
import concourse.tile as tile
from concourse import bass_utils

# ---- hardcoded problem constants ----
B, S, NNODE, CIN = 64, 512, 21, 2
D1, D2 = 64, 128
G = B * S                      # 32768 graphs
N_CORES = 8
G_CORE = G // N_CORES          # 4096 graphs per core
CHUNK = 512                    # graphs per pipeline chunk
N_CHUNKS = G_CORE // CHUNK
K1 = NNODE * CIN + 1           # 43 contraction rows (42 feats + ones row)
KT = 11                        # 1408 / 128 k-tiles for stage 2
M1 = KT * 128                  # 1408 = 1344 (s,f) cols + 1 bias col + 63 pad

EDGES = np.array(
    [[0, 1], [1, 2], [2, 3], [3, 4], [0, 5], [5, 6], [6, 7], [7, 8],
     [0, 9], [9, 10], [10, 11], [11, 12], [0, 13], [13, 14], [14, 15],
     [15, 16], [0, 17], [17, 18], [18, 19], [19, 20], [5, 9], [9, 13],
     [13, 17]], dtype=np.int64)


def fold_weights(W1, b1, W2, b2):
    """Fold adjacency, mean-pool and biases into two dense operands."""
    W1 = np.asarray(W1, np.float32)
    b1 = np.asarray(b1, np.float32)
    W2 = np.asarray(W2, np.float32)
    b2 = np.asarray(b2, np.float32)
    A = np.eye(NNODE, dtype=np.float32)
    A[EDGES[:, 1], EDGES[:, 0]] = 1.0
    deg = A.sum(axis=1)
    dis = 1.0 / np.sqrt(deg)
    a_norm = dis[:, None] * A * dis[None, :]          # [t, s] float32
    m = a_norm.mean(axis=0)                           # [21], all > 0

    # tw[(s',c), (s,f)] = m[s] * a_norm[s, s'] * W1[c, f]
    tw = np.zeros((K1, M1), np.float32)
    blk = np.einsum("s,st,cf->tcsf", m, a_norm, W1)   # [s'=t, c, s, f]
    tw[: NNODE * CIN, : NNODE * D1] = blk.reshape(NNODE * CIN, NNODE * D1)
    tw[K1 - 1, : NNODE * D1] = (m[:, None] * b1[None, :]).reshape(-1)
    tw[K1 - 1, NNODE * D1] = 1.0                      # relu's to constant 1

    w2full = np.zeros((M1, D2), np.float32)
    w2full[: NNODE * D1] = np.tile(W2, (NNODE, 1))
    w2full[NNODE * D1] = b2                           # rides the const-1 row
    # device tile is [128, KT*128] with pass-k slice [:, k*128:(k+1)*128]
    w2r = np.ascontiguousarray(
        w2full.reshape(KT, 128, D2).transpose(1, 0, 2).reshape(128, KT * D2))
    return tw.astype(np.float16), w2r.astype(np.float16)


def build_bass():
    f16 = mybir.dt.float16
    f32 = mybir.dt.float32
    nc = bass.Bass("TRN2", target_bir_lowering=False, debug=False)
    xt_d = nc.dram_tensor("xt", [K1, G_CORE], f16, kind="ExternalInput").ap()
    tw_d = nc.dram_tensor("tw", [K1, M1], f16, kind="ExternalInput").ap()
    w2r_d = nc.dram_tensor("w2r", [128, KT * 128], f16,
                           kind="ExternalInput").ap()
    out_d = nc.dram_tensor("out", [D2, G_CORE], f32, kind="ExternalOutput").ap()

    relu = mybir.ActivationFunctionType.Relu
    copyf = mybir.ActivationFunctionType.Copy

    ACT_KS = {0, 2, 4, 6, 8}          # relu k-tiles on ScalarE; rest on DVE
    SKEW = 4                          # MM3'(u) issued after MM1(u+SKEW)
    NU = N_CHUNKS * KT                # 88 pipeline units

    with tile.TileContext(nc) as tc:
        with (
            tc.tile_pool(name="w", bufs=1) as wpool,
            tc.tile_pool(name="x", bufs=N_CHUNKS) as xpool,
            tc.tile_pool(name="ra", bufs=3) as rapool,
            tc.tile_pool(name="rv", bufs=3) as rvpool,
            tc.tile_pool(name="o", bufs=N_CHUNKS) as opool,
            tc.tile_pool(name="pa", bufs=3, space="PSUM") as papool,
            tc.tile_pool(name="pv", bufs=3, space="PSUM") as pvpool,
            tc.tile_pool(name="po", bufs=2, space="PSUM") as popool,
        ):
            # per-k weight tiles -> fine-grained DMA deps, parallel queues
            tw_sb = []
            w2r_sb = []
            for k in range(KT):
                t = wpool.tile([K1, 128], f16, tag=f"tw{k}")
                nc.sync.dma_start(out=t, in_=tw_d[:, k * 128:(k + 1) * 128])
                tw_sb.append(t)
            for k in range(KT):
                t = wpool.tile([128, 128], f16, tag=f"w2r{k}")
                nc.sync.dma_start(out=t, in_=w2r_d[:, k * 128:(k + 1) * 128])
                w2r_sb.append(t)
            xt_sb = []
            for ch in range(N_CHUNKS):
                t = xpool.tile([K1, CHUNK], f16)
                nc.sync.dma_start(
                    out=t, in_=xt_d[:, ch * CHUNK:(ch + 1) * CHUNK])
                xt_sb.append(t)

            # HAM warmup: ~5us of dependency-free matmuls while DMAs land
            wu = wpool.tile([K1, CHUNK], f16, tag="wu")
            nc.gpsimd.memset(wu, 0.0)
            for _ in range(12):
                pt = papool.tile([128, CHUNK], f32, tag="pa")
                nc.tensor.matmul(pt, lhsT=wu[:, :128], rhs=wu,
                                 start=True, stop=True)

            out_ps = {}
            rts = {}

            def mm3(u):
                ch, k = divmod(u, KT)
                nc.tensor.matmul(
                    out_ps[ch],
                    lhsT=w2r_sb[k],
                    rhs=rts.pop(u),
                    start=(k == 0), stop=(k == KT - 1),
                    skip_group_check=True,
                )
                if k == KT - 1:
                    ot = opool.tile([D2, CHUNK], f32)
                    if ch % 2 == 0:
                        nc.scalar.activation(out=ot, in_=out_ps.pop(ch),
                                             func=copyf)
                    else:
                        nc.vector.tensor_copy(out=ot, in_=out_ps.pop(ch))
                    nc.sync.dma_start(
                        out=out_d[:, ch * CHUNK:(ch + 1) * CHUNK], in_=ot)

            for u in range(NU):
                ch, k = divmod(u, KT)
                if k == 0:
                    out_ps[ch] = popool.tile([D2, CHUNK], f32, tag="po",
                                             name=f"ops{ch}")
                on_act = k in ACT_KS
                pt = (papool if on_act else pvpool).tile(
                    [128, CHUNK], f32, tag="pa" if on_act else "pv")
                nc.tensor.matmul(
                    pt, lhsT=tw_sb[k], rhs=xt_sb[ch],
                    start=True, stop=True,
                )
                rt = (rapool if on_act else rvpool).tile(
                    [128, CHUNK], f16, tag="ra" if on_act else "rv")
                if on_act:
                    nc.scalar.activation(out=rt, in_=pt, func=relu)
                else:
                    nc.vector.tensor_scalar_max(out=rt, in0=pt, scalar1=0.0)
                rts[u] = rt
                if u >= SKEW:
                    mm3(u - SKEW)
            for u in range(NU - SKEW, NU):
                mm3(u)
    _rebalance_matmul_waits(nc)
    return nc


def _rebalance_matmul_waits(nc):
    """Walrus' TPB ISA structs accept only one sync-wait per instruction on
    the compute engines, but Tile can attach several (PE completion-order +
    cross-engine WAR + DMA). Keep one wait on the instruction and move the
    excess onto the immediately-preceding Ldweights (for matmuls) or onto
    freshly inserted same-engine NoOps — those execute just before on the
    same in-order queue, so waiting there is the same or stronger ordering."""
    import bass_rust
    import concourse.mybir as mybir

    exempt = {"InstEventSemaphore", "InstUnconditionalBranch",
              "InstCall", "InstISA", "InstNoOp"}
    nop_ctr = [0]
    for fn in nc.m.functions:
        for blk in fn.blocks:
            insts = list(blk.instructions)
            out = []
            pending_free_ldw = None
            for inst in insts:
                tn = type(inst).__name__
                if tn == "InstLdweights":
                    si = inst.sync_info
                    if si is None or len(si.on_wait) == 0:
                        pending_free_ldw = inst
                    out.append(inst)
                    continue
                si = inst.sync_info
                nw = len(si.on_wait) if si is not None else 0
                if tn in exempt or nw <= 1:
                    out.append(inst)
                    if tn == "InstMatmult":
                        pending_free_ldw = None
                    continue
                waits = list(si.on_wait)
                moved, kept = waits[:-1], waits[-1:]
                if tn == "InstMatmult" and pending_free_ldw is not None \
                        and len(moved) == 1:
                    c = pending_free_ldw
                    csi = c.sync_info
                    c.sync_info = bass_rust.SyncInfo(
                        on_wait=moved,
                        on_update=list(csi.on_update) if csi else [])
                else:
                    for w in moved:
                        nop_ctr[0] += 1
                        nop = mybir.InstNoOp(
                            name=f"I-waitnop-{nop_ctr[0]}", ins=[], outs=[])
                        nop.engine = inst.engine
                        nop.sync_info = bass_rust.SyncInfo(
                            on_wait=[w], on_update=[])
                        out.append(nop)
                inst.sync_info = bass_rust.SyncInfo(
                    on_wait=kept, on_update=list(si.on_update))
                out.append(inst)
                if tn == "InstMatmult":
                    pending_free_ldw = None
            if len(out) != len(insts):
                blk.instructions = out


_NC_CACHE = None


def _get_nc():
    global _NC_CACHE
    if _NC_CACHE is None:
        _NC_CACHE = build_bass()
    return _NC_CACHE


def make_in_maps(hand_landmarks, W1, b1, W2, b2):
    tw, w2r = fold_weights(W1, b1, W2, b2)
    x = np.asarray(hand_landmarks, np.float32).reshape(G, NNODE * CIN)
    xt = np.empty((K1, G), np.float16)
    xt[: NNODE * CIN] = x.T
    xt[K1 - 1] = 1.0
    return [
        {
            "xt": np.ascontiguousarray(xt[:, i * G_CORE:(i + 1) * G_CORE]),
            "tw": tw,
            "w2r": w2r,
        }
        for i in range(N_CORES)
    ]


def gather_out(results):
    full = np.concatenate([results[i]["out"] for i in range(N_CORES)], axis=1)
    return np.ascontiguousarray(full.T).reshape(B, S, D2).astype(np.float32)


def run(in_maps, trace=False, **kw):
    res = bass_utils.run_bass_kernel_spmd(
        _get_nc(), in_maps, core_ids=list(range(N_CORES)), trace=trace, **kw)
    return res


def kernel(hand_landmarks, W1, b1, W2, b2):
    in_maps = make_in_maps(hand_landmarks, W1, b1, W2, b2)
    res = run(in_maps)
    return gather_out(res.results)
